# revision 1
# baseline (speedup 1.0000x reference)
"""DiscreteOptionActor Trainium2 kernel.

Computes, for each sample b, logits = MLP_{option[b]}(obs[b]) where each of the
16 options has its own 3-layer MLP (128 -> 256 -> 256 -> 18, ReLU).

Strategy (MoE routing):
  - Host groups samples by option (argsort) and shards options across the 8
    cores: core k handles options 2k and 2k+1. Only the selected option's trunk
    is computed per sample (16x less compute than the dense reference).
  - Per (core, option) the gathered rows are padded to PAD=4352 and stored
    transposed (feature-major [128, PAD]) so matmuls run with features on
    partitions and samples on the free (moving) dimension.
  - Device: per option, L1/L2/L3 matmuls in float32r (full-rate PE mode),
    fused bias+ReLU from PSUM on ScalarE/VectorE, output logits^T [18, PAD].
  - Host scatters results back to original row order.
"""

import numpy as np

B, OBS, OPT, H1, H2, A = 65536, 128, 16, 256, 256, 18
NCORES = 8
OPC = OPT // NCORES  # options per core = 2
PAD = 4352  # padded rows per option (multiple of 128; max count ~4216)

_CACHE = {}


def _blocks():
    out = []
    st = 0
    while st < PAD:
        nb = min(512, PAD - st)
        out.append((st, nb))
        st += nb
    return out


def _build_program():
    import concourse.bass as bass
    import concourse.bacc as bacc
    import concourse.mybir as mybir
    import concourse.tile as tile

    f32 = mybir.dt.float32
    f32r = mybir.dt.float32r
    AF = mybir.ActivationFunctionType
    ALU = mybir.AluOpType

    f16 = mybir.dt.float16
    nc = bacc.Bacc(None, target_bir_lowering=False, debug=False)
    # xt (the dominant DMA stream) and w1 travel as fp16: 11 mantissa bits,
    # same order of rounding as float32r, at half the bytes.
    xt = nc.declare_dram_parameter("xt", [OPC, OBS, PAD], f16, isOutput=False)
    w1 = nc.declare_dram_parameter("w1", [OPC, OBS, H1], f16, isOutput=False)
    # w2/w3 pre-chunked on host to [o, p, k, n]: element [o,p,k,n] = W[o][k*128+p, n]
    w2 = nc.declare_dram_parameter("w2", [OPC, 128, 2, H2], f32r, isOutput=False)
    w3 = nc.declare_dram_parameter("w3", [OPC, 128, 2, A], f32r, isOutput=False)
    # biases host-transposed to [o, p, c]: element [o,p,c] = b[o][c*128+p]
    b1 = nc.declare_dram_parameter("b1", [OPC, 128, 2], f32, isOutput=False)
    b2 = nc.declare_dram_parameter("b2", [OPC, 128, 2], f32, isOutput=False)
    out = nc.declare_dram_parameter("out", [OPC, A, PAD], f16, isOutput=True)

    # pair-sized chunks: 4 x 1024 + 1 x 256 (PAD = 4352)
    pairs = []
    st = 0
    while st < PAD:
        nb = min(1024, PAD - st)
        pairs.append((st, nb))
        st += nb

    def halves(nb):
        out = []
        h = 0
        while h < nb:
            w = min(512, nb - h)
            out.append((h, w))
            h += w
        return out

    with tile.TileContext(nc) as tc:
        with (
            tc.tile_pool(name="wp", bufs=2) as wp,
            tc.tile_pool(name="xp", bufs=2) as xp,
            tc.tile_pool(name="hp1", bufs=3) as hp1,
            tc.tile_pool(name="hp2", bufs=3) as hp2,
            tc.tile_pool(name="op", bufs=3) as op,
            tc.tile_pool(name="dxp", bufs=1) as dxp,
            tc.tile_pool(name="psp", bufs=4, space=bass.MemorySpace.PSUM) as psp,
        ):
            # smaller leading chunks so L1 can start as early as possible
            xchunks = [(0, 512), (512, 512), (1024, 1024), (2048, 1024), (3072, 1024), (4096, 256)]
            assert sum(nb for _, nb in xchunks) == PAD

            # PE warm-up first: memset + dummy matmuls start the HAM ramp
            # at the earliest possible moment; ACT table load pulled early too
            dummy = dxp.tile([128, 64], f32, tag="dummy")
            dummy_o = dxp.tile([128, 1], f32, tag="dummy_o")
            nc.gpsimd.memset(dummy[:], 0.0)
            nc.scalar.activation(dummy_o[:], dummy[:, 0:1], AF.Relu, bias=0.0)

            xtts = []
            for o in range(OPC):
                xtt = xp.tile([OBS, PAD], f16, tag="xt", name=f"xtt{o}")
                xtts.append(xtt)
            nc.gpsimd.dma_start(
                xtts[0][:, 0:512], xt[0][:, 0:512]
            )
            for _ in range(10):
                pw = psp.tile([128, 1024], f32, tag="ps")
                nc.tensor.matmul(
                    pw[:64, :64], dummy[:, :], dummy[:, :], start=True, stop=True
                )

            for o in range(OPC):
                xtt = xtts[o]
                w1t = wp.tile([OBS, H1], f16, tag="w1")
                b1t = wp.tile([128, 2], f32, tag="b1")
                w2t = wp.tile([128, 2, H2], f32r, tag="w2")
                b2t = wp.tile([128, 2], f32, tag="b2")
                w3t = wp.tile([128, 2, A], f32r, tag="w3")
                # tiny w1/b1/b2 first on sync (every drain waits on the
                # biases); xt chunks next (even->gpsimd, odd->sync); big
                # w2/w3 last
                nc.sync.dma_start(w1t[:], w1[o])
                nc.sync.dma_start(b1t[:], b1[o])
                nc.sync.dma_start(b2t[:], b2[o])
                for ci, (st, nb) in enumerate(xchunks):
                    if o == 0 and st == 0:
                        continue  # already issued above
                    deng = nc.gpsimd if ci % 2 == 0 else nc.sync
                    deng.dma_start(xtt[:, st : st + nb], xt[o][:, st : st + nb])
                nc.sync.dma_start(w2t[:], w2[o])
                nc.sync.dma_start(w3t[:], w3[o])

                h1c = [hp1.tile([128, PAD], f32r, tag="h1", name=f"h1c{_c}") for _c in range(2)]
                h2c = [hp2.tile([128, PAD], f32r, tag="h2", name=f"h2c{_c}") for _c in range(2)]

                # Drains alternate ACT/DVE so the PSUM rotation is never
                # limited by a single drain engine in any phase.
                def relu_drain(dst, ps_ap, bias_ap, use_act):
                    if use_act:
                        nc.scalar.activation(dst, ps_ap, AF.Relu, bias=bias_ap)
                    else:
                        nc.vector.tensor_scalar(
                            dst, ps_ap, bias_ap, 0.0, ALU.add, ALU.max
                        )

                # L1: h1^T = relu(W1^T x + b1). For option 0 (the startup
                # region, where the drain-latency chain gates the pipeline
                # fill) use 512-col granularity; 1024 elsewhere.
                di = 0
                l1_blocks = pairs
                for st, nb in l1_blocks:
                    for c in range(2):
                        ps = psp.tile([128, 1024], f32, tag="ps")
                        for h, w in halves(nb):
                            nc.tensor.matmul(
                                ps[:, h : h + w],
                                w1t[:, c * 128 : (c + 1) * 128],
                                xtt[:, st + h : st + h + w],
                                start=True,
                                stop=True,
                            )
                        relu_drain(
                            h1c[c][:, st : st + nb],
                            ps[:, :nb],
                            b1t[:, c : c + 1],
                            di % 2 == 0,
                        )
                        di += 1

                # L2 + L3 interleaved per pair: L3(p) runs right after L2(p)
                # so L3 drains spread across the phase instead of clustering.
                for st, nb in pairs:
                    for m in range(2):
                        ps = psp.tile([128, 1024], f32, tag="ps")
                        for h, w in halves(nb):
                            for k in range(2):
                                nc.tensor.matmul(
                                    ps[:, h : h + w],
                                    w2t[:, k, m * 128 : (m + 1) * 128],
                                    h1c[k][:, st + h : st + h + w],
                                    start=(k == 0),
                                    stop=(k == 1),
                                )
                        relu_drain(
                            h2c[m][:, st : st + nb],
                            ps[:, :nb],
                            b2t[:, m : m + 1],
                            di % 2 == 0,
                        )
                        di += 1

                    outp = op.tile([A, 1024], f16, tag="out")
                    ps = psp.tile([A, 1024], f32, tag="ps")
                    for h, w in halves(nb):
                        for k in range(2):
                            nc.tensor.matmul(
                                ps[:, h : h + w],
                                w3t[:, k, :],
                                h2c[k][:, st + h : st + h + w],
                                start=(k == 0),
                                stop=(k == 1),
                            )
                    if nb < 1024 or di % 2 == 0:
                        nc.scalar.activation(outp[:, :nb], ps[:, :nb], AF.Copy)
                    else:
                        nc.vector.tensor_copy(outp[:, :nb], ps[:, :nb])
                    di += 1
                    eng = nc.sync if di % 2 == 0 else nc.gpsimd
                    eng.dma_start(out[o][:, st : st + nb], outp[:, :nb])
    nc.compile()
    return nc



PAIRS = [(0, 1024), (1024, 1024), (2048, 1024), (3072, 1024), (4096, 256)]
XCHUNKS = [(0, 512), (512, 512), (1024, 1024), (2048, 1024), (3072, 1024), (4096, 256)]
N_WARM = 10


def _halves(nb):
    out = []
    h = 0
    while h < nb:
        w = min(512, nb - h)
        out.append((h, w))
        h += w
    return out


def _chunk_hi(st, nb):
    """Index of the last xt chunk overlapping columns [st, st+nb)."""
    hi = 0
    for ci, (cst, cnb) in enumerate(XCHUNKS):
        if cst < st + nb:
            hi = ci
    return hi


def _build_raw():
    import concourse.bass as bass
    import concourse.bacc as bacc
    import concourse.mybir as mybir

    f32 = mybir.dt.float32
    f32r = mybir.dt.float32r
    f16 = mybir.dt.float16
    AF = mybir.ActivationFunctionType
    ALU = mybir.AluOpType

    nc = bacc.Bacc(None, target_bir_lowering=False, debug=False)
    xt = nc.declare_dram_parameter("xt", [OPC, OBS, PAD], f16, isOutput=False)
    w1 = nc.declare_dram_parameter("w1", [OPC, OBS, H1], f16, isOutput=False)
    w2 = nc.declare_dram_parameter("w2", [OPC, 128, 2, H2], f32r, isOutput=False)
    w3 = nc.declare_dram_parameter("w3", [OPC, 128, 2, A], f32r, isOutput=False)
    b1 = nc.declare_dram_parameter("b1", [OPC, 128, 2], f32, isOutput=False)
    b2 = nc.declare_dram_parameter("b2", [OPC, 128, 2], f32, isOutput=False)
    out = nc.declare_dram_parameter("out", [OPC, A, PAD], f16, isOutput=True)

    # --- on-chip tensors (no reuse across options -> no WAR hazards) ---
    xts = [nc.alloc_sbuf_tensor(f"xts{o}", [OBS, PAD], f16) for o in range(OPC)]
    w1s = [nc.alloc_sbuf_tensor(f"w1s{o}", [OBS, H1], f16) for o in range(OPC)]
    w2s = [nc.alloc_sbuf_tensor(f"w2s{o}", [128, 2, H2], f32r) for o in range(OPC)]
    w3s = [nc.alloc_sbuf_tensor(f"w3s{o}", [128, 2, A], f32r) for o in range(OPC)]
    b1s = [nc.alloc_sbuf_tensor(f"b1s{o}", [128, 2], f32) for o in range(OPC)]
    b2s = [nc.alloc_sbuf_tensor(f"b2s{o}", [128, 2], f32) for o in range(OPC)]
    h1s = [
        [nc.alloc_sbuf_tensor(f"h1_{o}_{c}", [128, PAD], f32r) for c in range(2)]
        for o in range(OPC)
    ]
    h2s = [
        [nc.alloc_sbuf_tensor(f"h2_{o}_{m}", [128, PAD], f32r) for m in range(2)]
        for o in range(OPC)
    ]
    osb1 = nc.alloc_sbuf_tensor("osb", [A, PAD], f16)
    osb = [osb1 for _ in range(OPC)]
    dummy = nc.alloc_sbuf_tensor("warm_dummy", [128, 64], f32)
    dummy_o = nc.alloc_sbuf_tensor("warm_dummy_o", [128, 1], f32)

    pss = [nc.alloc_psum_tensor(f"ps{s}", [128, 1024], f32) for s in range(4)]

    # --- semaphores ---
    ws = nc.alloc_semaphore("warm_sem")
    xsem = [[nc.alloc_semaphore(f"x{o}_{ci}") for ci in range(len(XCHUNKS))]
            for o in range(OPC)]
    wname = ["w1", "b1", "b2", "w2", "w3"]
    wsem = [{n: nc.alloc_semaphore(f"wt{o}_{n}") for n in wname} for o in range(OPC)]
    fd = [nc.alloc_semaphore(f"fd{s}") for s in range(4)]
    prog = {}
    for o in range(OPC):
        for key in ("h1a", "h1v", "h2a", "h2v", "oa", "ov"):
            prog[(key, o)] = nc.alloc_semaphore(f"{key}{o}")
    odsems = [nc.alloc_semaphore(f"od{i}") for i in range(2 * len(PAIRS))]
    od_by_engine = {"sync": [], "gps": []}

    # --- static schedule ---
    # op descriptors collected per engine, then emitted inside nc.Block
    pe_ops = []   # list of (waits, mm_list, inc_sem) ; mm = (slot, h, w, lhs_fn, rhs_fn, start, stop, out_part)
    act_ops = []  # (waits, kind, args...)
    dve_ops = []
    sync_ops = []  # DMA issues in order: ("dma", waits, out_ap_fn, in_ap_fn, inc_sem, inc_val)
    gps_ops = []

    fill_count = [0, 0, 0, 0]   # completed-fill counts per slot (for fd thresholds)
    slot_prev_drain = [None, None, None, None]  # (sem, count) of previous tenant
    fill_idx = 0
    prog_count = {k: 0 for k in prog}

    pe_last_wait = {}  # sem name -> last waited value (skip redundant waits)

    def pe_wait(waits, sem, val):
        key = sem.name if hasattr(sem, "name") else id(sem)
        if pe_last_wait.get(key, -1) < val:
            waits.append((sem, val))
            pe_last_wait[key] = val

    def emit_fill(data_waits, mms, out_part, drain_engine, drain_emit):
        """mms: list of (h, w, lhs_fn, rhs_fn, start, stop). drain_emit is
        (kind-args tuple) appended to act_ops/dve_ops with computed waits."""
        nonlocal fill_idx
        s = fill_idx % 4
        fill_idx += 1
        waits = []
        if slot_prev_drain[s] is not None:
            sem, cnt = slot_prev_drain[s]
            pe_wait(waits, sem, cnt)
        for sem, val in data_waits:
            pe_wait(waits, sem, val)
        pe_ops.append((waits, s, mms, out_part, fd[s]))
        fill_count[s] += 1
        fd_thresh = fill_count[s]
        # drain
        kind, dst_fn, bias, psem_key, extra_waits = drain_emit
        o = psem_key[1]
        sem = prog[psem_key]
        prog_count[psem_key] += 1
        cnt = prog_count[psem_key]
        drain_waits = [(fd[s], fd_thresh)] + extra_waits
        op = (drain_waits, kind, s, dst_fn, bias, sem)
        if drain_engine == "act":
            act_ops.append(op)
        else:
            dve_ops.append(op)
        slot_prev_drain[s] = (sem, cnt)
        return prog_count[psem_key]

    # DMA issue schedule. Even xt chunks on gpsimd; sync gets the tiny
    # weight/bias tensors FIRST (drains need b1/b2 early), interleaved with
    # the odd xt chunks so nothing big delays them.
    for o in range(OPC):
        def xdma(ci, o=o):
            cst, cnb = XCHUNKS[ci]
            return ("dma", [],
                    (lambda: xts[o].ap()[:, cst:cst + cnb]),
                    (lambda: xt[o][:, cst:cst + cnb]),
                    xsem[o][ci], 16)

        def wdma(n, o=o):
            src = {"w1": w1, "b1": b1, "b2": b2, "w2": w2, "w3": w3}[n]
            dst = {"w1": w1s, "b1": b1s, "b2": b2s, "w2": w2s, "w3": w3s}[n]
            return ("dma", [], (lambda: dst[o].ap()[:]), (lambda: src[o]),
                    wsem[o][n], 16)

        gps_ops.extend([xdma(0), xdma(2), xdma(4)])
        sync_ops.extend([wdma("w1"), wdma("b1"), wdma("b2"), xdma(1),
                         xdma(3), xdma(5), wdma("w2"), wdma("w3")])

    # interleave: we want xt chunks of o=0 issued before o=1 weights etc.
    # simple reorder: stable sort is already in insertion order per engine; fine.

    out_dma_counts = {"sync": 0, "gps": 0}
    di = 0          # global drain-engine alternation counter
    l1_thr = {}     # (o,p) -> (h1a,h1v) progress counts after that pair's L1 drains
    l2_thr = {}     # (o,p) -> (h2a,h2v)
    osb_war = {}    # st -> (od_sem, completion threshold) of o0's out-DMA

    def emit_l1(o, p):
        nonlocal di
        st, nb = PAIRS[p]
        for c in range(2):
            data_waits = [(xsem[o][ci], 16) for ci in range(_chunk_hi(st, nb) + 1)]
            data_waits.append((wsem[o]["w1"], 16))
            mms = []
            for h, w in _halves(nb):
                mms.append((
                    h, w,
                    (lambda o=o, c=c: w1s[o].ap()[:, c * 128:(c + 1) * 128]),
                    (lambda o=o, st=st, h=h, w=w: xts[o].ap()[:, st + h:st + h + w]),
                    True, True,
                ))
            eng = "act" if di % 2 == 0 else "dve"
            emit_fill(
                data_waits, mms, 128, eng,
                ("relu",
                 (lambda o=o, c=c, st=st, nb=nb: h1s[o][c].ap()[:, st:st + nb]),
                 (lambda o=o, c=c: b1s[o].ap()[:, c:c + 1]),
                 ("h1a" if eng == "act" else "h1v", o),
                 [(wsem[o]["b1"], 16)]),
            )
            di += 1
        l1_thr[(o, p)] = (prog_count[("h1a", o)], prog_count[("h1v", o)])

    def emit_l2(o, p):
        nonlocal di
        st, nb = PAIRS[p]
        na, nv = l1_thr[(o, p)]
        for m in range(2):
            data_waits = [(wsem[o]["w2"], 16)]
            if na:
                data_waits.append((prog[("h1a", o)], na))
            if nv:
                data_waits.append((prog[("h1v", o)], nv))
            mms = []
            for h, w in _halves(nb):
                for k in range(2):
                    mms.append((
                        h, w,
                        (lambda o=o, k=k, m=m: w2s[o].ap()[:, k, m * 128:(m + 1) * 128]),
                        (lambda o=o, k=k, st=st, h=h, w=w: h1s[o][k].ap()[:, st + h:st + h + w]),
                        k == 0, k == 1,
                    ))
            eng = "act" if di % 2 == 0 else "dve"
            emit_fill(
                data_waits, mms, 128, eng,
                ("relu",
                 (lambda o=o, m=m, st=st, nb=nb: h2s[o][m].ap()[:, st:st + nb]),
                 (lambda o=o, m=m: b2s[o].ap()[:, m:m + 1]),
                 ("h2a" if eng == "act" else "h2v", o),
                 [(wsem[o]["b2"], 16)]),
            )
            di += 1
        l2_thr[(o, p)] = (prog_count[("h2a", o)], prog_count[("h2v", o)])

    def emit_l3(o, p):
        nonlocal di
        st, nb = PAIRS[p]
        na, nv = l2_thr[(o, p)]
        data_waits = [(wsem[o]["w3"], 16)]
        if na:
            data_waits.append((prog[("h2a", o)], na))
        if nv:
            data_waits.append((prog[("h2v", o)], nv))
        mms = []
        for h, w in _halves(nb):
            for k in range(2):
                mms.append((
                    h, w,
                    (lambda o=o, k=k: w3s[o].ap()[:, k, :]),
                    (lambda o=o, k=k, st=st, h=h, w=w: h2s[o][k].ap()[:, st + h:st + h + w]),
                    k == 0, k == 1,
                ))
        eng = "act" if (nb < 1024 or di % 2 == 0) else "dve"
        war_waits = []
        if o > 0 and st in osb_war:
            war_waits.append(osb_war[st])
        cnt = emit_fill(
            data_waits, mms, A, eng,
            ("copy",
             (lambda o=o, st=st, nb=nb: osb[o].ap()[:, st:st + nb]),
             None,
             ("oa" if eng == "act" else "ov", o),
             war_waits),
        )
        di += 1
        issue = "sync" if di % 2 == 0 else "gps"
        odsem = odsems[len(od_by_engine["sync"]) + len(od_by_engine["gps"])]
        od_by_engine[issue].append(odsem)
        osem = prog[("oa" if eng == "act" else "ov", o)]
        dma_op = ("dma", [(osem, cnt)],
                  (lambda o=o, st=st, nb=nb: out[o][:, st:st + nb]),
                  (lambda o=o, st=st, nb=nb: osb[o].ap()[:, st:st + nb]),
                  odsem, 16)
        (sync_ops if issue == "sync" else gps_ops).append(dma_op)
        out_dma_counts[issue] += 1
        if o == 0:
            osb_war[st] = (odsem, 16)

    # Global software pipeline: L1 runs two pair-groups ahead of L2;
    # L3 trails its pair's L2 by one group, across both options.
    l1q = [(o, p) for o in range(OPC) for p in range(len(PAIRS))]
    l2q = list(l1q)
    l3q = []
    emit_l1(*l1q.pop(0))
    emit_l1(*l1q.pop(0))
    for (o, p) in l2q:
        if l1q:
            emit_l1(*l1q.pop(0))
        emit_l2(o, p)
        l3q.append((o, p))
        if len(l3q) > 1:
            emit_l3(*l3q.pop(0))
    while l3q:
        emit_l3(*l3q.pop(0))

    # --- emit engine programs ---
    with nc.Block() as block:

        @block.gpsimd
        def _(eng):
            nc.gpsimd.memset(dummy.ap()[:], 0.0).then_inc(ws, 1)
            for op in gps_ops:
                _, waits, dst_fn, src_fn, sem, val = op
                for wsem_, wval in waits:
                    eng.wait_ge(wsem_, wval)
                eng.dma_start(out=dst_fn(), in_=src_fn()).then_inc(sem, val)
            for s_ in od_by_engine["gps"]:
                eng.wait_ge(s_, 16)

        @block.sync
        def _(eng):
            for op in sync_ops:
                _, waits, dst_fn, src_fn, sem, val = op
                for wsem_, wval in waits:
                    eng.wait_ge(wsem_, wval)
                eng.dma_start(out=dst_fn(), in_=src_fn()).then_inc(sem, val)
            for s_ in od_by_engine["sync"]:
                eng.wait_ge(s_, 16)

        @block.tensor
        def _(eng):
            eng.wait_ge(ws, 1)
            for _i in range(N_WARM):
                nc.tensor.matmul(
                    pss[0].ap()[:64, :64], dummy.ap()[:, :], dummy.ap()[:, :],
                    start=True, stop=True,
                )
            for waits, s, mms, out_part, fdsem in pe_ops:
                for wsem_, wval in waits:
                    eng.wait_ge(wsem_, wval)
                for j, (h, w, lhs_fn, rhs_fn, stt, stp) in enumerate(mms):
                    inst = nc.tensor.matmul(
                        pss[s].ap()[:out_part, h:h + w],
                        lhs_fn(), rhs_fn(), start=stt, stop=stp,
                    )
                    if j == len(mms) - 1:
                        inst.then_inc(fdsem, 1)

        @block.scalar
        def _(eng):
            # dummy activation pulls the Relu ACT table load into the prologue
            eng.wait_ge(ws, 1)
            nc.scalar.activation(dummy_o.ap()[:], dummy.ap()[:, 0:1], AF.Relu, bias=0.0)
            for waits, kind, s, dst_fn, bias_fn, sem in act_ops:
                for wsem_, wval in waits:
                    eng.wait_ge(wsem_, wval)
                dst = dst_fn()
                nbv = dst.shape[-1]
                src = pss[s].ap()[: dst.shape[0], :nbv]
                if kind == "relu":
                    inst = nc.scalar.activation(dst, src, AF.Relu, bias=bias_fn())
                else:
                    inst = nc.scalar.activation(dst, src, AF.Copy)
                inst.then_inc(sem, 1)

        @block.vector
        def _(eng):
            for waits, kind, s, dst_fn, bias_fn, sem in dve_ops:
                for wsem_, wval in waits:
                    eng.wait_ge(wsem_, wval)
                dst = dst_fn()
                nbv = dst.shape[-1]
                src = pss[s].ap()[: dst.shape[0], :nbv]
                if kind == "relu":
                    inst = nc.vector.tensor_scalar(
                        dst, src, bias_fn(), 0.0, ALU.add, ALU.max
                    )
                else:
                    inst = nc.vector.tensor_copy(dst, src)
                inst.then_inc(sem, 1)

    nc.compile()
    return nc




def _build_tile_pipe():
    """TileContext builder with globally software-pipelined emission order:
    L1 runs two pair-groups ahead of L2, L3 trails by one group, across both
    options — Tile turns emission order into scheduling priority, so the PE
    stream interleaves layers instead of draining phase-by-phase."""
    import concourse.bass as bass
    import concourse.bacc as bacc
    import concourse.mybir as mybir
    import concourse.tile as tile

    f32 = mybir.dt.float32
    f32r = mybir.dt.float32r
    AF = mybir.ActivationFunctionType
    ALU = mybir.AluOpType

    nc = bacc.Bacc(None, target_bir_lowering=False, debug=False)
    xt = nc.declare_dram_parameter("xt", [OPC, OBS, PAD], f32r, isOutput=False)
    w1 = nc.declare_dram_parameter("w1", [OPC, OBS, H1], f32r, isOutput=False)
    w2 = nc.declare_dram_parameter("w2", [OPC, 128, 2, H2], f32r, isOutput=False)
    w3 = nc.declare_dram_parameter("w3", [OPC, 128, 2, A], f32r, isOutput=False)
    b1 = nc.declare_dram_parameter("b1", [OPC, 128, 2], f32, isOutput=False)
    b2 = nc.declare_dram_parameter("b2", [OPC, 128, 2], f32, isOutput=False)
    out = nc.declare_dram_parameter("out", [OPC, A, PAD], f16, isOutput=True)

    pairs = PAIRS
    xchunks = XCHUNKS

    with tile.TileContext(nc) as tc:
        with (
            tc.tile_pool(name="wp", bufs=2) as wp,
            tc.tile_pool(name="xp", bufs=2) as xp,
            tc.tile_pool(name="hp1", bufs=4) as hp1,
            tc.tile_pool(name="hp2", bufs=3) as hp2,
            tc.tile_pool(name="op", bufs=3) as op,
            tc.tile_pool(name="dxp", bufs=1) as dxp,
            tc.tile_pool(name="psp", bufs=4, space=bass.MemorySpace.PSUM) as psp,
        ):
            # dummy warm-up + early ACT table load
            dummy = dxp.tile([128, 64], f32, tag="dummy")
            dummy_o = dxp.tile([128, 1], f32, tag="dummy_o")
            nc.gpsimd.memset(dummy[:], 0.0)
            nc.scalar.activation(dummy_o[:], dummy[:, 0:1], AF.Relu, bias=0.0)
            for _ in range(18):
                pw = psp.tile([128, 1024], f32, tag="ps")
                nc.tensor.matmul(
                    pw[:64, :64], dummy[:, :], dummy[:, :], start=True, stop=True
                )

            # all input DMAs up front, per option: even xt chunks on gpsimd,
            # small weights first then odd chunks on sync
            xtts, w1ts, b1ts, w2ts, b2ts, w3ts = [], [], [], [], [], []
            for o in range(OPC):
                xtt = xp.tile([OBS, PAD], f32r, tag="xt", name=f"xtt{o}")
                xtts.append(xtt)
                w1t = wp.tile([OBS, H1], f32r, tag="w1", name=f"w1t{o}")
                b1t = wp.tile([128, 2], f32, tag="b1", name=f"b1t{o}")
                w2t = wp.tile([128, 2, H2], f32r, tag="w2", name=f"w2t{o}")
                b2t = wp.tile([128, 2], f32, tag="b2", name=f"b2t{o}")
                w3t = wp.tile([128, 2, A], f32r, tag="w3", name=f"w3t{o}")
                w1ts.append(w1t); b1ts.append(b1t); w2ts.append(w2t)
                b2ts.append(b2t); w3ts.append(w3t)
                nc.sync.dma_start(w1t[:], w1[o])
                nc.sync.dma_start(b1t[:], b1[o])
                nc.sync.dma_start(b2t[:], b2[o])
            # xt chunks interleaved across options by need-time: o1's lead
            # chunks land before o0's tail so L1(o1) starts without a gap
            gorder = [(0, 0), (0, 2), (1, 0), (0, 4), (1, 2), (1, 4)]
            sorder = [(0, 1), (0, 3), (1, 1), (0, 5), (1, 3), (1, 5)]
            for o, ci in gorder:
                cst, cnb = xchunks[ci]
                nc.gpsimd.dma_start(xtts[o][:, cst:cst + cnb], xt[o][:, cst:cst + cnb])
            for o, ci in sorder:
                cst, cnb = xchunks[ci]
                nc.sync.dma_start(xtts[o][:, cst:cst + cnb], xt[o][:, cst:cst + cnb])
            for o in range(OPC):
                nc.sync.dma_start(w2ts[o][:], w2[o])
                nc.sync.dma_start(w3ts[o][:], w3[o])

            h1cs = {}
            h2cs = {}
            state = {"di": 0}

            def relu_drain(dst, ps_ap, bias_ap):
                use_act = state["di"] % 2 == 0
                state["di"] += 1
                if use_act:
                    nc.scalar.activation(dst, ps_ap, AF.Relu, bias=bias_ap)
                else:
                    nc.vector.tensor_scalar(dst, ps_ap, bias_ap, 0.0, ALU.add, ALU.max)

            def emit_l1(o, p):
                if o not in h1cs:
                    h1cs[o] = [
                        hp1.tile([128, PAD], f32r, tag="h1", name=f"h1_{o}_{c}")
                        for c in range(2)
                    ]
                st, nb = pairs[p]
                for c in range(2):
                    ps = psp.tile([128, 1024], f32, tag="ps")
                    h = 0
                    while h < nb:
                        w = min(512, nb - h)
                        nc.tensor.matmul(
                            ps[:, h:h + w],
                            w1ts[o][:, c * 128:(c + 1) * 128],
                            xtts[o][:, st + h:st + h + w],
                            start=True, stop=True,
                        )
                        h += w
                    relu_drain(h1cs[o][c][:, st:st + nb], ps[:, :nb], b1ts[o][:, c:c + 1])

            def emit_l2(o, p):
                if o not in h2cs:
                    h2cs[o] = [
                        hp2.tile([128, PAD], f32r, tag="h2", name=f"h2_{o}_{m}")
                        for m in range(2)
                    ]
                st, nb = pairs[p]
                for m in range(2):
                    ps = psp.tile([128, 1024], f32, tag="ps")
                    h = 0
                    while h < nb:
                        w = min(512, nb - h)
                        for k in range(2):
                            nc.tensor.matmul(
                                ps[:, h:h + w],
                                w2ts[o][:, k, m * 128:(m + 1) * 128],
                                h1cs[o][k][:, st + h:st + h + w],
                                start=(k == 0), stop=(k == 1),
                            )
                        h += w
                    relu_drain(h2cs[o][m][:, st:st + nb], ps[:, :nb], b2ts[o][:, m:m + 1])

            def emit_l3(o, p):
                st, nb = pairs[p]
                outp = op.tile([A, 1024], f16, tag="out")
                ps = psp.tile([A, 1024], f32, tag="ps")
                h = 0
                while h < nb:
                    w = min(512, nb - h)
                    for k in range(2):
                        nc.tensor.matmul(
                            ps[:, h:h + w],
                            w3ts[o][:, k, :],
                            h2cs[o][k][:, st + h:st + h + w],
                            start=(k == 0), stop=(k == 1),
                        )
                    h += w
                if nb < 1024 or state["di"] % 2 == 0:
                    nc.scalar.activation(outp[:, :nb], ps[:, :nb], AF.Copy)
                else:
                    nc.vector.tensor_copy(outp[:, :nb], ps[:, :nb])
                state["di"] += 1
                eng = nc.sync if state["di"] % 2 == 0 else nc.gpsimd
                eng.dma_start(out[o][:, st:st + nb], outp[:, :nb])

            l1q = [(o, p) for o in range(OPC) for p in range(len(pairs))]
            l2q = list(l1q)
            l3q = []
            emit_l1(*l1q.pop(0))
            emit_l1(*l1q.pop(0))
            for (o, p) in l2q:
                if l1q:
                    emit_l1(*l1q.pop(0))
                emit_l2(o, p)
                l3q.append((o, p))
                if len(l3q) > 1:
                    emit_l3(*l3q.pop(0))
            while l3q:
                emit_l3(*l3q.pop(0))
    nc.compile()
    return nc

def _get_program():
    # The layer-major TileContext build (_build_program) measured most
    # consistently (62-64us). _build_tile_pipe (pipelined emission) and
    # _build_raw (manual semaphores) measured equal-or-worse and are kept
    # for reference.
    if "nc" not in _CACHE:
        _CACHE["nc"] = _build_raw()
    return _CACHE["nc"]


def _prep(inputs):
    obs = np.ascontiguousarray(np.asarray(inputs["obs"], dtype=np.float32))
    option = np.asarray(inputs["option"]).astype(np.int64, copy=False)
    W1 = np.asarray(inputs["W1"], dtype=np.float32)
    b1 = np.asarray(inputs["b1"], dtype=np.float32)
    W2 = np.asarray(inputs["W2"], dtype=np.float32)
    b2 = np.asarray(inputs["b2"], dtype=np.float32)
    W3 = np.asarray(inputs["W3"], dtype=np.float32)
    b3 = np.asarray(inputs["b3"], dtype=np.float32)

    order = np.argsort(option, kind="stable")
    sorted_opt = option[order]
    starts = np.searchsorted(sorted_opt, np.arange(OPT + 1))
    idx_per_opt = [order[starts[o] : starts[o + 1]] for o in range(OPT)]

    in_maps = []
    for core in range(NCORES):
        sl = slice(core * OPC, (core + 1) * OPC)
        xt = np.zeros((OPC, OBS, PAD), np.float16)
        for lo in range(OPC):
            idx = idx_per_opt[core * OPC + lo][:PAD]
            xt[lo, :, : len(idx)] = obs[idx].T
        w2c = W2[sl].reshape(OPC, 2, 128, H2).transpose(0, 2, 1, 3)
        w3c = W3[sl].reshape(OPC, 2, 128, A).transpose(0, 2, 1, 3)
        b1c = b1[sl].reshape(OPC, 2, 128).transpose(0, 2, 1)
        b2c = b2[sl].reshape(OPC, 2, 128).transpose(0, 2, 1)
        in_maps.append(
            {
                "xt": xt,
                "w1": np.ascontiguousarray(W1[sl].astype(np.float16)),
                "w2": np.ascontiguousarray(w2c),
                "w3": np.ascontiguousarray(w3c),
                "b1": np.ascontiguousarray(b1c),
                "b2": np.ascontiguousarray(b2c),
            }
        )
    host = dict(obs=obs, W1=W1, b1=b1, W2=W2, b2=b2, W3=W3, b3=b3)
    return in_maps, idx_per_opt, host


def _unshard(results, idx_per_opt, host):
    out_full = np.empty((B, 1, A), np.float32)
    for core in range(NCORES):
        res = results[core]["out"]  # [OPC, A, PAD]
        for lo in range(OPC):
            o = core * OPC + lo
            idx = idx_per_opt[o]
            n = min(len(idx), PAD)
            out_full[idx[:n], 0, :] = res[lo, :, :n].T + host["b3"][o]
            if len(idx) > n:  # overflow beyond PAD: compute on host (rare/never)
                rows = host["obs"][idx[n:]]
                h = np.maximum(rows @ host["W1"][o] + host["b1"][o], 0.0)
                h = np.maximum(h @ host["W2"][o] + host["b2"][o], 0.0)
                out_full[idx[n:], 0, :] = h @ host["W3"][o] + host["b3"][o]
    return out_full


def run(inputs, trace=False, **spmd_kwargs):
    """Run the kernel; returns (output, BassKernelResults)."""
    from concourse.bass_utils import run_bass_kernel_spmd

    in_maps, idx_per_opt, host = _prep(inputs)
    nc = _get_program()
    try:
        br = run_bass_kernel_spmd(
            nc, in_maps, list(range(NCORES)), trace=trace, **spmd_kwargs
        )
    except Exception:
        # transient device/runtime hiccups have been observed once per
        # session; rebuild the program and retry once
        _CACHE.clear()
        nc = _get_program()
        br = run_bass_kernel_spmd(
            nc, in_maps, list(range(NCORES)), trace=trace, **spmd_kwargs
        )
    return _unshard(br.results, idx_per_opt, host), br


def kernel(**inputs):
    out, _ = run(inputs)
    return out



# revision 6
# speedup vs baseline: 1.1784x; 1.1784x over previous
"""DiscreteOptionActor Trainium2 kernel (v2).

Computes, for each sample b, logits = MLP_{option[b]}(obs[b]) where each of the
16 options has its own 3-layer MLP (128 -> 256 -> 256 -> 18, ReLU).

Strategy (MoE routing):
  - Host groups samples by option (argsort); core k handles options 2k, 2k+1.
  - Per (core, option) the gathered rows are padded to PAD=4352, stored
    transposed (feature-major [128, PAD]) in fp16.
  - All weights packed into ONE byte tensor per option (single DMA + sem).
  - Device: 3-layer MLP per option, fp16 matmuls (1 col/cycle), fused
    bias+ReLU drains alternating ScalarE/VectorE, fp16 logits^T out.
  - Host scatters results back and adds b3.
"""

import numpy as np

B, OBS, OPT, H1, H2, A = 65536, 128, 16, 256, 256, 18
NCORES = 8
OPC = OPT // NCORES  # options per core = 2
PAD = 4352

_CACHE = {}

# 1024-col drain granularity
PAIRS = [(0, 1024), (1024, 1024), (2048, 1024), (3072, 1024), (4096, 256)]
# xt DMA chunks (cols): sized so the first L1 fills can start early
XCHUNKS = [(0, 512), (512, 512), (1024, 1024), (2048, 1024), (3072, 1280)]
N_WARM = 18
WARM_N = 128

# packed weight layout (bytes per partition):
#   w1 fp16 [256]        bytes    0:512    (f16 idx   0:256) lhsT c: [c*128:(c+1)*128]
#   w2 fp16 [2,256]      bytes  512:1536   (f16 idx 256:768) (k,m): [256+k*256+m*128 : +128]
#   w3 fp16 [2,18]       bytes 1536:1608   (f16 idx 768:804) k: [768+k*18 : +18]
#   b1 f32  [2]          bytes 1608:1616
#   b2 f32  [2]          bytes 1616:1624
WPK_BYTES = 1624


def _halves(nb):
    out = []
    h = 0
    while h < nb:
        w = min(512, nb - h)
        out.append((h, w))
        h += w
    return out


def _chunk_hi(st, nb):
    """Index of the last xt chunk overlapping columns [st, st+nb)."""
    hi = 0
    for ci, (cst, cnb) in enumerate(XCHUNKS):
        if cst < st + nb:
            hi = ci
    return hi


def _build_v2():
    import concourse.bass as bass
    import concourse.bacc as bacc
    import concourse.mybir as mybir

    f32 = mybir.dt.float32
    f16 = mybir.dt.float16
    AF = mybir.ActivationFunctionType
    ALU = mybir.AluOpType

    nc = bacc.Bacc(None, target_bir_lowering=False, debug=False)
    xt = nc.declare_dram_parameter("xt", [OPC, OBS, PAD], f16, isOutput=False)
    wpk = nc.declare_dram_parameter("wpk", [OPC, 128, WPK_BYTES], mybir.dt.uint8,
                                    isOutput=False)
    out = nc.declare_dram_parameter("out", [OPC, A, PAD], f16, isOutput=True)

    # --- on-chip tensors ---
    xts = [nc.alloc_sbuf_tensor(f"xts{o}", [OBS, PAD], f16) for o in range(OPC)]
    wps = [nc.alloc_sbuf_tensor(f"wps{o}", [128, WPK_BYTES], mybir.dt.uint8)
           for o in range(OPC)]
    h1s = [[nc.alloc_sbuf_tensor(f"h1_{o}_{c}", [128, PAD], f16) for c in range(2)]
           for o in range(OPC)]
    h2s = [[nc.alloc_sbuf_tensor(f"h2_{o}_{m}", [128, PAD], f16) for m in range(2)]
           for o in range(OPC)]
    osbs = [nc.alloc_sbuf_tensor(f"osb{o}", [A, PAD], f16) for o in range(OPC)]
    dummy = nc.alloc_sbuf_tensor("warm_dummy", [128, WARM_N], f16)
    dummy_o = nc.alloc_sbuf_tensor("warm_dummy_o", [128, 1], f32)

    pss = [nc.alloc_psum_tensor(f"ps{s}", [128, 1024], f32) for s in range(4)]

    def wf16(o):
        return wps[o].ap().bitcast(f16)

    def w1_ap(o, c):
        return wf16(o)[:, c * 128:(c + 1) * 128]

    def w2_ap(o, k, m):
        base = 256 + k * 256 + m * 128
        return wf16(o)[:, base:base + 128]

    def w3_ap(o, k):
        base = 768 + k * 18
        return wf16(o)[:, base:base + 18]

    def b_ap(o, which, c):
        # which: 0 -> b1, 1 -> b2
        bb = wps[o].ap()[:, 1608 + 8 * which: 1616 + 8 * which].bitcast(f32)
        return bb[:, c:c + 1]

    # --- semaphores ---
    gsem = nc.alloc_semaphore("gate")        # clock-start gate for gpsimd
    ws = nc.alloc_semaphore("warm_sem")
    xsem = [[nc.alloc_semaphore(f"x{o}_{ci}") for ci in range(len(XCHUNKS))]
            for o in range(OPC)]
    wsem = [nc.alloc_semaphore(f"wt{o}") for o in range(OPC)]
    fd = [nc.alloc_semaphore(f"fd{s}") for s in range(4)]
    prog = {}
    for o in range(OPC):
        for key in ("h1a", "h1v", "h2a", "h2v", "oa", "ov"):
            prog[(key, o)] = nc.alloc_semaphore(f"{key}{o}")
    odsem = nc.alloc_semaphore("od")

    # --- static schedule containers ---
    pe_ops = []
    act_ops = []
    dve_ops = []
    sync_ops = []
    gps_ops = []

    fill_count = [0, 0, 0, 0]
    slot_prev_drain = [None, None, None, None]
    fill_idx = 0
    prog_count = {k: 0 for k in prog}
    od_count = [0]

    pe_last_wait = {}

    def pe_wait(waits, sem, val):
        key = sem.name if hasattr(sem, "name") else id(sem)
        if pe_last_wait.get(key, -1) < val:
            waits.append((sem, val))
            pe_last_wait[key] = val

    def emit_fill(data_waits, mms, out_part, drain_engine, drain_emit):
        nonlocal fill_idx
        s = fill_idx % 4
        fill_idx += 1
        waits = []
        if slot_prev_drain[s] is not None:
            sem, cnt = slot_prev_drain[s]
            pe_wait(waits, sem, cnt)
        for sem, val in data_waits:
            pe_wait(waits, sem, val)
        pe_ops.append((waits, s, mms, out_part, fd[s]))
        fill_count[s] += 1
        fd_thresh = fill_count[s]
        kind, dst_fn, bias, psem_key, extra_waits = drain_emit
        sem = prog[psem_key]
        prog_count[psem_key] += 1
        cnt = prog_count[psem_key]
        drain_waits = [(fd[s], fd_thresh)] + extra_waits
        op = (drain_waits, kind, s, dst_fn, bias, sem)
        if drain_engine == "act":
            act_ops.append(op)
        else:
            dve_ops.append(op)
        slot_prev_drain[s] = (sem, cnt)
        return prog_count[psem_key]

    # --- input DMA schedule ---
    # sync: xt0c0, wpk0, xt0c1, xt0c2, wpk1, xt1c0, xt1c1
    # gps:  xt0c3, xt0c4, xt1c2, xt1c3, xt1c4
    def xdma(o, ci):
        cst, cnb = XCHUNKS[ci]
        return ("dma", [],
                (lambda o=o, cst=cst, cnb=cnb: xts[o].ap()[:, cst:cst + cnb]),
                (lambda o=o, cst=cst, cnb=cnb: xt[o][:, cst:cst + cnb]),
                xsem[o][ci], 16, None)

    def wdma(o):
        return ("dma", [],
                (lambda o=o: wps[o].ap()[:]),
                (lambda o=o: wpk[o]),
                wsem[o], 16, None)

    sync_ops.extend([
        ("dma_gate", [], (lambda: xts[0].ap()[:, 0:512]),
         (lambda: xt[0][:, 0:512]), xsem[0][0], 16, gsem),
        wdma(0), xdma(0, 1), xdma(0, 2),
        wdma(1), xdma(1, 0), xdma(1, 1),
    ])
    gps_ops.extend([xdma(0, 3), xdma(0, 4), xdma(1, 2), xdma(1, 3), xdma(1, 4)])

    di = 0
    l1_thr = {}
    l2_thr = {}

    def emit_l1(o, p):
        nonlocal di
        st, nb = PAIRS[p]
        for c in range(2):
            mms = []
            data_waits = [(wsem[o], 16)]
            for h, w in _halves(nb):
                hi = _chunk_hi(st + h, w)
                mms.append((
                    h, w,
                    (lambda o=o, c=c: w1_ap(o, c)),
                    (lambda o=o, st=st, h=h, w=w: xts[o].ap()[:, st + h:st + h + w]),
                    True, True, [(xsem[o][hi], 16)],
                ))
            eng = "act" if di % 2 == 0 else "dve"
            emit_fill(
                data_waits, mms, 128, eng,
                ("relu",
                 (lambda o=o, c=c, st=st, nb=nb: h1s[o][c].ap()[:, st:st + nb]),
                 (lambda o=o, c=c: b_ap(o, 0, c)),
                 ("h1a" if eng == "act" else "h1v", o),
                 []),
            )
            di += 1
        l1_thr[(o, p)] = (prog_count[("h1a", o)], prog_count[("h1v", o)])

    def emit_l2(o, p):
        nonlocal di
        st, nb = PAIRS[p]
        na, nv = l1_thr[(o, p)]
        for m in range(2):
            data_waits = [(wsem[o], 16)]
            if na:
                data_waits.append((prog[("h1a", o)], na))
            if nv:
                data_waits.append((prog[("h1v", o)], nv))
            mms = []
            for h, w in _halves(nb):
                for k in range(2):
                    mms.append((
                        h, w,
                        (lambda o=o, k=k, m=m: w2_ap(o, k, m)),
                        (lambda o=o, k=k, st=st, h=h, w=w: h1s[o][k].ap()[:, st + h:st + h + w]),
                        k == 0, k == 1, None,
                    ))
            eng = "act" if di % 2 == 0 else "dve"
            emit_fill(
                data_waits, mms, 128, eng,
                ("relu",
                 (lambda o=o, m=m, st=st, nb=nb: h2s[o][m].ap()[:, st:st + nb]),
                 (lambda o=o, m=m: b_ap(o, 1, m)),
                 ("h2a" if eng == "act" else "h2v", o),
                 []),
            )
            di += 1
        l2_thr[(o, p)] = (prog_count[("h2a", o)], prog_count[("h2v", o)])

    def emit_l3(o, p):
        nonlocal di
        st, nb = PAIRS[p]
        na, nv = l2_thr[(o, p)]
        data_waits = [(wsem[o], 16)]
        if na:
            data_waits.append((prog[("h2a", o)], na))
        if nv:
            data_waits.append((prog[("h2v", o)], nv))
        mms = []
        for h, w in _halves(nb):
            for k in range(2):
                mms.append((
                    h, w,
                    (lambda o=o, k=k: w3_ap(o, k)),
                    (lambda o=o, k=k, st=st, h=h, w=w: h2s[o][k].ap()[:, st + h:st + h + w]),
                    k == 0, k == 1, None,
                ))
        eng = "act" if (nb < 1024 or di % 2 == 0) else "dve"
        cnt = emit_fill(
            data_waits, mms, A, eng,
            ("copy",
             (lambda o=o, st=st, nb=nb: osbs[o].ap()[:, st:st + nb]),
             None,
             ("oa" if eng == "act" else "ov", o),
             []),
        )
        di += 1
        osem = prog[("oa" if eng == "act" else "ov", o)]
        od_count[0] += 1
        dma_op = ("dma", [(osem, cnt)],
                  (lambda o=o, st=st, nb=nb: out[o][:, st:st + nb]),
                  (lambda o=o, st=st, nb=nb: osbs[o].ap()[:, st:st + nb]),
                  odsem, 16, None)
        sync_ops.append(dma_op)

    # global software pipeline: L1 two pair-groups ahead; L3 trails by one
    l1q = [(o, p) for o in range(OPC) for p in range(len(PAIRS))]
    l2q = list(l1q)
    l3q = []
    emit_l1(*l1q.pop(0))
    emit_l1(*l1q.pop(0))
    for (o, p) in l2q:
        if l1q:
            emit_l1(*l1q.pop(0))
        emit_l2(o, p)
        l3q.append((o, p))
        if len(l3q) > 1:
            emit_l3(*l3q.pop(0))
    while l3q:
        emit_l3(*l3q.pop(0))

    n_od = od_count[0]

    # --- emit engine programs ---
    with nc.Block() as block:

        @block.sync
        def _(eng):
            for op in sync_ops:
                kind, waits, dst_fn, src_fn, sem, val, extra_inc = op
                for wsem_, wval in waits:
                    eng.wait_ge(wsem_, wval)
                inst = eng.dma_start(out=dst_fn(), in_=src_fn())
                inst.then_inc(sem, val)
                if extra_inc is not None:
                    # gate release for gpsimd clock-start
                    eng.sem_inc(extra_inc, 1)
            eng.wait_ge(odsem, 16 * n_od)

        @block.gpsimd
        def _(eng):
            eng.wait_ge(gsem, 1)
            nc.gpsimd.memset(dummy.ap()[:], 0.0).then_inc(ws, 1)
            for op in gps_ops:
                kind, waits, dst_fn, src_fn, sem, val, extra_inc = op
                for wsem_, wval in waits:
                    eng.wait_ge(wsem_, wval)
                eng.dma_start(out=dst_fn(), in_=src_fn()).then_inc(sem, val)

        @block.tensor
        def _(eng):
            eng.wait_ge(ws, 1)
            for _i in range(N_WARM):
                nc.tensor.matmul(
                    pss[0].ap()[:128, :WARM_N], dummy.ap()[:, :], dummy.ap()[:, :],
                    start=True, stop=True,
                )
            mm_seen = {}
            for waits, s, mms, out_part, fdsem in pe_ops:
                for wsem_, wval in waits:
                    eng.wait_ge(wsem_, wval)
                for j, (h, w, lhs_fn, rhs_fn, stt, stp, mwaits) in enumerate(mms):
                    if mwaits:
                        for wsem_, wval in mwaits:
                            key = wsem_.name if hasattr(wsem_, "name") else id(wsem_)
                            if mm_seen.get(key, -1) < wval:
                                eng.wait_ge(wsem_, wval)
                                mm_seen[key] = wval
                    inst = nc.tensor.matmul(
                        pss[s].ap()[:out_part, h:h + w],
                        lhs_fn(), rhs_fn(), start=stt, stop=stp,
                    )
                    if j == len(mms) - 1:
                        inst.then_inc(fdsem, 1)

        @block.scalar
        def _(eng):
            eng.wait_ge(ws, 1)
            nc.scalar.activation(dummy_o.ap()[:], dummy.ap()[:, 0:1], AF.Relu, bias=0.0)
            for waits, kind, s, dst_fn, bias_fn, sem in act_ops:
                for wsem_, wval in waits:
                    eng.wait_ge(wsem_, wval)
                dst = dst_fn()
                nbv = dst.shape[-1]
                src = pss[s].ap()[: dst.shape[0], :nbv]
                if kind == "relu":
                    inst = nc.scalar.activation(dst, src, AF.Relu, bias=bias_fn())
                else:
                    inst = nc.scalar.activation(dst, src, AF.Copy)
                inst.then_inc(sem, 1)

        @block.vector
        def _(eng):
            for waits, kind, s, dst_fn, bias_fn, sem in dve_ops:
                for wsem_, wval in waits:
                    eng.wait_ge(wsem_, wval)
                dst = dst_fn()
                nbv = dst.shape[-1]
                src = pss[s].ap()[: dst.shape[0], :nbv]
                if kind == "relu":
                    inst = nc.vector.tensor_scalar(
                        dst, src, bias_fn(), 0.0, ALU.add, ALU.max
                    )
                else:
                    inst = nc.vector.tensor_copy(dst, src)
                inst.then_inc(sem, 1)

    nc.compile()
    return nc


def _get_program():
    if "nc" not in _CACHE:
        _CACHE["nc"] = _build_v2()
    return _CACHE["nc"]


def _prep(inputs):
    obs = np.ascontiguousarray(np.asarray(inputs["obs"], dtype=np.float32))
    option = np.asarray(inputs["option"]).astype(np.int64, copy=False)
    W1 = np.asarray(inputs["W1"], dtype=np.float32)
    b1 = np.asarray(inputs["b1"], dtype=np.float32)
    W2 = np.asarray(inputs["W2"], dtype=np.float32)
    b2 = np.asarray(inputs["b2"], dtype=np.float32)
    W3 = np.asarray(inputs["W3"], dtype=np.float32)
    b3 = np.asarray(inputs["b3"], dtype=np.float32)

    order = np.argsort(option, kind="stable")
    sorted_opt = option[order]
    starts = np.searchsorted(sorted_opt, np.arange(OPT + 1))
    idx_per_opt = [order[starts[o]: starts[o + 1]] for o in range(OPT)]

    in_maps = []
    for core in range(NCORES):
        xtc = np.zeros((OPC, OBS, PAD), np.float16)
        wpk = np.zeros((OPC, 128, WPK_BYTES), np.uint8)
        for lo in range(OPC):
            o = core * OPC + lo
            idx = idx_per_opt[o][:PAD]
            xtc[lo, :, : len(idx)] = obs[idx].T
            # pack weights: per partition p of 128
            w1p = np.ascontiguousarray(W1[o].astype(np.float16))  # [128, 256]
            w2p = np.ascontiguousarray(
                W2[o].reshape(2, 128, H2).transpose(1, 0, 2).astype(np.float16)
            ).reshape(128, -1)                                    # [128, 512]
            w3p = np.ascontiguousarray(
                W3[o].reshape(2, 128, A).transpose(1, 0, 2).astype(np.float16)
            ).reshape(128, -1)                                    # [128, 36]
            b1p = np.ascontiguousarray(b1[o].reshape(2, 128).T.astype(np.float32))
            b2p = np.ascontiguousarray(b2[o].reshape(2, 128).T.astype(np.float32))
            buf = np.concatenate([
                w1p.view(np.uint8),
                w2p.view(np.uint8),
                w3p.view(np.uint8),
                b1p.view(np.uint8),
                b2p.view(np.uint8),
            ], axis=1)
            assert buf.shape == (128, WPK_BYTES), buf.shape
            wpk[lo] = buf
        in_maps.append({"xt": xtc, "wpk": wpk})
    host = dict(obs=obs, W1=W1, b1=b1, W2=W2, b2=b2, W3=W3, b3=b3)
    return in_maps, idx_per_opt, host


def _unshard(results, idx_per_opt, host):
    out_full = np.empty((B, 1, A), np.float32)
    for core in range(NCORES):
        res = results[core]["out"]  # [OPC, A, PAD]
        for lo in range(OPC):
            o = core * OPC + lo
            idx = idx_per_opt[o]
            n = min(len(idx), PAD)
            out_full[idx[:n], 0, :] = res[lo, :, :n].T + host["b3"][o]
            if len(idx) > n:  # overflow beyond PAD: compute on host (rare/never)
                rows = host["obs"][idx[n:]]
                h = np.maximum(rows @ host["W1"][o] + host["b1"][o], 0.0)
                h = np.maximum(h @ host["W2"][o] + host["b2"][o], 0.0)
                out_full[idx[n:], 0, :] = h @ host["W3"][o] + host["b3"][o]
    return out_full


def run(inputs, trace=False, **spmd_kwargs):
    """Run the kernel; returns (output, BassKernelResults)."""
    from concourse.bass_utils import run_bass_kernel_spmd

    in_maps, idx_per_opt, host = _prep(inputs)
    nc = _get_program()
    try:
        br = run_bass_kernel_spmd(
            nc, in_maps, list(range(NCORES)), trace=trace, **spmd_kwargs
        )
    except Exception:
        _CACHE.clear()
        nc = _get_program()
        br = run_bass_kernel_spmd(
            nc, in_maps, list(range(NCORES)), trace=trace, **spmd_kwargs
        )
    return _unshard(br.results, idx_per_opt, host), br


def kernel(**inputs):
    out, _ = run(inputs)
    return out
